# revision 19
# baseline (speedup 1.0000x reference)
"""AttentionPooling (segment softmax-mean) Trainium2 kernel, v3.

pooled[g] = mean over graph g of softmax_g(score)-weighted x rows, where
score_i = tanh(x_i @ w1 + b1) @ w2 + b2 and graph ids (batch) are sorted.

Strategy: 8 cores, graphs split contiguously and node-balanced. All device
data fp16 (halves HBM traffic; PE streams 1 col/cycle vs 4 for fp32).

Per 512-node batch: one contiguous slab DMA brings x (chunk-major) plus a
host-built one-hot node->graph mask M. PE transposes x chunks via REGULAR
matmuls (x stationary, fp16 identity moving; fast FWL weight loads, warms
HAM, unlike transpose-mode). DVE+GPSIMD each copy one xT half PSUM->SBUF
(cast to fp16). PE MLP -> ACT tanh -> PE scores as COLUMNS (h-chunk
stationary, w2 moving) so ACT exp runs on [128, NCH] (cheap) and feeds the
DVE mask multiply A = M * e directly — no DRAM bounce. PE segment matmul
pool[d, g] += x_chunk.T @ A accumulates in one persistent PSUM bank across
ALL batches (has_written accumulate semantics; a start=True zeroing matmul
initializes the bank), so graph pieces spanning chunk/batch boundaries just
add up.

The per-batch stages are software-pipelined across iterations (stage lags
0/1/2/3/4) so the in-order PE queue never stalls on the DVE/ACT chain —
without this, PE idles ~1us/batch and HAM re-throttles it to 1.2 GHz.

Denominators (sum of e per graph) and the final 1/(denom*count) scaling are
applied host-side from the exported e columns. The Bass program is JIT-
specialized per call: chunk graph windows from the actual (sorted) batch
vector are compile-time constants; per-core programs build in parallel.
"""
import numpy as np

N_CORES = 8
D = 256
H = 128
NB = 512           # nodes per device batch
NCH = NB // 128    # 128-node chunks per batch


def _plan_shards(batch, num_graphs):
    counts = np.bincount(batch, minlength=num_graphs).astype(np.int64)
    starts = np.concatenate([[0], np.cumsum(counts)])  # [B+1]
    n = int(starts[-1])
    cuts = [0]
    for c in range(1, N_CORES):
        target = n * c // N_CORES
        g = int(np.searchsorted(starts, target, side="left"))
        g = max(cuts[-1] + 1, min(g, num_graphs - (N_CORES - c)))
        cuts.append(g)
    cuts.append(num_graphs)
    shards = []
    for c in range(N_CORES):
        g0, g1 = cuts[c], cuts[c + 1]
        n0, n1 = int(starts[g0]), int(starts[g1])
        shards.append(dict(g0=g0, g1=g1, n0=n0, n1=n1,
                           counts=counts[g0:g1],
                           gstarts=starts[g0:g1 + 1] - n0))
    return shards


def _windows_for_shard(sh):
    """Per (batch, chunk): (g_lo, gw) local graph window covering the chunk's
    nodes (gw=0 for all-padding chunks). Returns nb, npad, G, GW, windows."""
    nodes = sh["n1"] - sh["n0"]
    nb = (nodes + NB - 1) // NB
    npad = nb * NB
    G = sh["g1"] - sh["g0"]
    gstarts = sh["gstarts"]
    windows = []
    GW = 1
    for b in range(nb):
        row = []
        for c in range(NCH):
            lo = b * NB + c * 128
            hi = min(lo + 128, nodes)
            if hi <= lo:
                row.append((0, 0))
                continue
            glo = max(0, int(np.searchsorted(gstarts, lo, side="right")) - 1)
            ghi = max(0, int(np.searchsorted(gstarts, hi - 1, side="right")) - 1)
            gw = ghi - glo + 1
            GW = max(GW, gw)
            row.append((glo, gw))
        windows.append(row)
    return nb, npad, G, GW, windows


def _build_slab(sh, nb, npad, G, GW, windows, x_core_f16, batch_local):
    """slab[b, p, c, 0:D] = x row (b*NB + c*128 + p); slab[b, p, c, D+j] =
    one-hot mask of local graph windows[b][c][0] + j."""
    nodes = sh["n1"] - sh["n0"]
    slab = np.zeros((nb, 128, NCH, D + GW), dtype=np.float16)
    xp = np.zeros((npad, D), dtype=np.float16)
    xp[:nodes] = x_core_f16
    slab[:, :, :, :D] = xp.reshape(nb, NCH, 128, D).transpose(0, 2, 1, 3)
    ids = np.full(npad, -1, dtype=np.int64)
    ids[:nodes] = batch_local
    ids = ids.reshape(nb, NCH, 128)
    j = np.arange(GW)
    for b in range(nb):
        for c in range(NCH):
            glo, gw = windows[b][c]
            if gw:
                slab[b, :, c, D:] = (ids[b, c][:, None] == glo + j)
    return slab.reshape(nb, 128, NCH * (D + GW))


def _build_core_program(sh, b2f):
    import concourse.bass as bass
    import concourse.bacc as bacc
    import concourse.mybir as mybir
    import concourse.tile as tile

    nb, npad, G, GW, windows = _windows_for_shard(sh)
    f16, f32 = mybir.dt.float16, mybir.dt.float32
    AF = mybir.ActivationFunctionType

    nc = bacc.Bacc("TRN2", target_bir_lowering=False, debug=False)
    slab_in = nc.declare_dram_parameter("slab", [nb, 128, NCH * (D + GW)],
                                        f16, isOutput=False)
    w1_in = nc.declare_dram_parameter("w1", [D, H], f16, isOutput=False)
    b1_in = nc.declare_dram_parameter("b1", [H, 1], f32, isOutput=False)
    w2_in = nc.declare_dram_parameter("w2", [H, 1], f16, isOutput=False)
    ident_in = nc.declare_dram_parameter("ident", [128, 128], f16,
                                         isOutput=False)
    out_p = nc.declare_dram_parameter("pooledT", [128, 2 * G], f32,
                                      isOutput=True)
    e_out = nc.declare_dram_parameter("e", [128, nb * NCH], f16, isOutput=True)

    with tile.TileContext(nc) as tc:
        with tc.tile_pool(name="const", bufs=1) as const, \
             tc.tile_pool(name="xp", bufs=10) as xp, \
             tc.tile_pool(name="xtp", bufs=3) as xtp, \
             tc.tile_pool(name="hp", bufs=4) as hp, \
             tc.tile_pool(name="apl", bufs=5) as apl, \
             tc.tile_pool(name="fin", bufs=1) as fin, \
             tc.tile_pool(name="ps_xt", bufs=4, space="PSUM") as ps_xt, \
             tc.tile_pool(name="ps_h", bufs=2, space="PSUM") as ps_h, \
             tc.tile_pool(name="ps_s", bufs=1, space="PSUM") as ps_s, \
             tc.tile_pool(name="ps_acc", bufs=1, space="PSUM") as ps_acc:

            # ---- constants ----
            ident = const.tile([128, 128], f16, tag="ident")
            nc.gpsimd.dma_start(out=ident, in_=ident_in[:, :])
            w1sb = const.tile([128, 2, H], f16, tag="w1sb")
            nc.gpsimd.dma_start(out=w1sb,
                                in_=w1_in.rearrange("(f p) h -> p f h", f=2))
            b1col = const.tile([H, 1], f32, tag="b1col")
            nc.gpsimd.dma_start(out=b1col, in_=b1_in[:, :])
            b2col = const.tile([128, 1], f32, tag="b2col")
            nc.vector.memset(b2col, b2f)
            w2sb = const.tile([H, 1], f16, tag="w2sb")
            nc.gpsimd.dma_start(out=w2sb, in_=w2_in[:, :])
            zeros = const.tile([128, 2 * G], f16, tag="zeros")
            nc.vector.memset(zeros, 0.0)
            # e columns accumulate here; exported once at the end so no
            # per-batch DMA sits on the critical cross-engine cycle.
            e_all = const.tile([128, nb * NCH], f16, tag="eall")

            # ---- persistent pooled accumulator: [d(128p), f*G + g] ----
            pacc = ps_acc.tile([128, 2 * G], f32, tag="pacc")
            # start=True zeroing matmul clears stale has_written bits.
            nc.tensor.matmul(pacc, zeros[:, 0:128], zeros[:, 0:2 * G],
                             start=True, stop=False, skip_group_check=True)

            n_seg = sum(1 for b in range(nb) for c in range(NCH)
                        if windows[b][c][1])
            seg_i = 0
            slab_t, xt_ps, xt_sb, h_sb, s_ps, e_cols, a_t = \
                {}, {}, {}, {}, {}, {}, {}

            def st_dma(b):
                t = xp.tile([128, NCH, D + GW], f16, tag="x", name=f"x{b}")
                slab_t[b] = t
                nc.sync.dma_start(
                    out=t,
                    in_=slab_in[b].rearrange("p (c e) -> p c e", c=NCH))

            def st0_transpose(b):
                t = slab_t[b]
                xt = [ps_xt.tile([128, NB], f32, tag="xt",
                                 name=f"xtp{b}_{f}") for f in range(2)]
                xt_ps[b] = xt
                for c in range(NCH):
                    for f in range(2):
                        nc.tensor.matmul(
                            xt[f][:, c * 128:(c + 1) * 128],
                            t[:, c, f * 128:(f + 1) * 128],
                            ident, start=True, stop=True)

            def st1_copy(b):
                xt = xt_ps.pop(b)
                sb = [xtp.tile([128, NB], f16, tag="xts",
                               name=f"xts{b}_{f}") for f in range(2)]
                xt_sb[b] = sb
                nc.vector.tensor_copy(sb[0], xt[0])
                nc.vector.tensor_copy(sb[1][:, 0:320], xt[1][:, 0:320])
                nc.scalar.copy(out=sb[1][:, 320:NB], in_=xt[1][:, 320:NB])

            def st2_mlp_tanh(b):
                sb = xt_sb.pop(b)
                h_ps = ps_h.tile([H, NB], f32, tag="h", name=f"h{b}")
                for f in range(2):
                    nc.tensor.matmul(h_ps, w1sb[:, f, :], sb[f],
                                     start=(f == 0), stop=(f == 1))
                hs = hp.tile([H, NB], f16, tag="hsb", name=f"hsb{b}")
                h_sb[b] = hs
                nc.scalar.activation(out=hs, in_=h_ps, func=AF.Tanh,
                                     bias=b1col, scale=1.0)

            def st3_scores_exp_a(b):
                hs = h_sb.pop(b)
                sp = ps_s.tile([128, NCH], f32, tag="s", name=f"s{b}")
                for c in range(NCH):
                    nc.tensor.matmul(sp[:, c:c + 1],
                                     hs[:, c * 128:(c + 1) * 128],
                                     w2sb, start=True, stop=True)
                ec = e_all[:, b * NCH:(b + 1) * NCH]
                nc.scalar.activation(out=ec, in_=sp, func=AF.Exp,
                                     bias=b2col, scale=1.0)
                at = apl.tile([128, NCH, GW], f16, tag="a", name=f"a{b}")
                a_t[b] = at
                t = slab_t[b]
                m_view = t[:, :, D:D + GW]
                e_b = bass.AP(tensor=ec.tensor, offset=ec.offset,
                              ap=[list(ec.ap[0]), list(ec.ap[1]), [0, GW]])
                nc.gpsimd.tensor_mul(out=at, in0=m_view, in1=e_b)

            def st4_segmm(b):
                nonlocal seg_i
                t, at = slab_t.pop(b), a_t.pop(b)
                for c in range(NCH):
                    glo, gw = windows[b][c]
                    if not gw:
                        continue
                    seg_i += 1
                    for f in range(2):
                        nc.tensor.matmul(
                            pacc[:, f * G + glo: f * G + glo + gw],
                            t[:, c, f * 128:(f + 1) * 128],
                            at[:, c, 0:gw],
                            start=False,
                            stop=(seg_i == n_seg and f == 1),
                            skip_group_check=True)

            # Reverse stage order per iteration: oldest-dependency stages
            # first so the in-order engine queues never head-block on a
            # younger stage's unmet dependency (EXP before TANH on ACT;
            # segmm first on PE). Slab DMA prefetches 2 iterations ahead.
            stages = (st_dma, st4_segmm, st3_scores_exp_a, st2_mlp_tanh,
                      st1_copy, st0_transpose)
            LAGS = (-3, 5, 4, 2, 1, 0)
            for it in range(-3, nb + 5):
                for st, lag in zip(stages, LAGS):
                    b = it - lag
                    if 0 <= b < nb:
                        st(b)

            # ---- export pooledT and e ----
            p_sb = fin.tile([128, 2 * G], f32, tag="psb")
            nc.vector.tensor_copy(p_sb, pacc)
            nc.sync.dma_start(out=out_p[:, :], in_=p_sb)
            nc.sync.dma_start(out=e_out[:, :], in_=e_all)

    nc.compile()
    return nc, nb, npad, G, GW, windows


def kernel(x, batch, num_graphs, w1, b1, w2, b2):
    from concourse.bass_utils import run_bass_kernel_spmd

    x = np.asarray(x, dtype=np.float32)
    batch = np.asarray(batch).astype(np.int64)
    B = int(num_graphs)
    w1f = np.asarray(w1, dtype=np.float16)
    b1f = np.asarray(b1, dtype=np.float32).reshape(H, 1)
    w2f = np.asarray(w2, dtype=np.float16).reshape(H, 1)
    b2f = float(np.asarray(b2, dtype=np.float32).reshape(-1)[0])

    shards = _plan_shards(batch, B)
    ident = np.eye(128, dtype=np.float16)
    out = np.zeros((B, D), dtype=np.float32)

    import concurrent.futures as cf

    def build(c):
        sh = shards[c]
        nc, nb, npad, G, GW, windows = _build_core_program(sh, b2f)
        slab = _build_slab(sh, nb, npad, G, GW, windows,
                           x[sh["n0"]: sh["n1"]].astype(np.float16),
                           batch[sh["n0"]: sh["n1"]] - sh["g0"])
        in_map = {"slab": slab, "w1": w1f, "b1": b1f,
                  "w2": w2f, "ident": ident}
        return c, nc, in_map

    with cf.ThreadPoolExecutor(max_workers=8) as ex:
        built = list(ex.map(build, range(N_CORES)))

    for c, nc, in_map in built:
        res = run_bass_kernel_spmd(nc, [in_map], [0])
        sh = shards[c]
        G = sh["g1"] - sh["g0"]
        nodes = sh["n1"] - sh["n0"]
        pooledT = res.results[0]["pooledT"].astype(np.float64)
        pooled = pooledT.reshape(128, 2, G).transpose(2, 1, 0).reshape(G, D)
        e_arr = res.results[0]["e"].astype(np.float64)  # [128, nb*NCH]
        nbv = e_arr.shape[1] // NCH
        e_lin = e_arr.reshape(128, nbv, NCH).transpose(1, 2, 0).reshape(-1)[:nodes]
        gstarts = sh["gstarts"]
        denom = np.add.reduceat(e_lin, np.minimum(gstarts[:-1], nodes - 1))
        seg_len = np.diff(gstarts)
        scale = denom * np.maximum(sh["counts"], 1.0)
        scale = np.where(seg_len == 0, 1.0, scale)
        pooled /= scale[:, None]
        pooled[seg_len == 0] = 0.0
        out[sh["g0"]: sh["g1"]] = pooled.astype(np.float32)
    return out
